# revision 1
# baseline (speedup 1.0000x reference)
"""Trainium2 kernel for nn_ActionPrompt.

Mathematical simplification of the reference model: both softmaxes are
taken over a length-1 axis (kv length 1), so their outputs are
identically 1.0.  That makes the entire stage-1 attention and the
stage-2 score computation dead code, and the output reduces exactly to

    out = (act @ Wv2 + bv2) @ Wo2 + bo2        # [A=50, D=1024]

independent of comb_fea.  (The reference's final mean over the
num_comb axis averages 1024 identical copies of o2.)

Distribution over 8 NeuronCores: shard the intermediate dimension
D=1024 into 8 chunks of 128.  Core i holds Wv2[:, ci] (col shard) and
Wo2[ci, :] (row shard) plus the full (replicated) act, computes

    partial_i = (act @ Wv2[:, ci] + bv2[ci]) @ Wo2[ci, :]   # [50, 1024]

and the host unshards by summing the 8 partials and adding bo2.  This
keeps all matmul FLOPs and all large-tensor traffic on the NeuronCores
while avoiding any on-device collective (whose ~10us floor would
dominate this tiny problem).
"""

import numpy as np

A = 50      # num actions
D = 1024    # embed dim
NCORES = 8
CHUNK = D // NCORES  # 128 cols/rows of the intermediate dim per core
KC = D // 128        # 8 K-chunks of 128 for the PE contraction

_CACHE = {}


def _build():
    """Build the Bass graph (same SPMD program for all 8 cores)."""
    import concourse.bacc as bacc
    import concourse.mybir as mybir
    from concourse.tile import TileContext

    f32 = mybir.dt.float32
    f32r = mybir.dt.float32r  # fp32 data, 4-xbus PE read mode (fast fp32 matmul)

    nc = bacc.Bacc("TRN2")

    # Per-core DRAM inputs (host pre-laid-out so every DMA is contiguous
    # with 128 partitions):
    #   a_in  [128, KC*A]  : actT chunk c in a_in[:, c*A:(c+1)*A] = act[:, 128c:128c+128].T
    #   w1_in [128, KC*128]: Wv2 col-shard, chunk c = Wv2[128c:128c+128, ci]
    #   w2_in [128, D]     : Wo2[ci, :] row shard (natural layout)
    #   b1_in [128, 1]     : bv2[ci]
    a_in = nc.dram_tensor("a_in", [128, KC * A], f32r, kind="ExternalInput")
    w1_in = nc.dram_tensor("w1_in", [128, KC * 128], f32r, kind="ExternalInput")
    w2_in = nc.dram_tensor("w2_in", [128, D], f32r, kind="ExternalInput")
    b1_in = nc.dram_tensor("b1_in", [128, 1], f32, kind="ExternalInput")
    out_d = nc.dram_tensor("out", [A, D], f32, kind="ExternalOutput")

    with TileContext(nc) as tc:
        with (
            tc.tile_pool(name="sb", bufs=1) as sb,
            tc.tile_pool(name="ps", bufs=1, space="PSUM") as ps,
        ):
            a_t = sb.tile([128, KC * A], f32r)
            w1_t = sb.tile([128, KC * 128], f32r)
            w2_t = sb.tile([128, D], f32r)
            b1_t = sb.tile([128, 1], f32)
            v2t = sb.tile([128, A], f32r)   # (act @ Wv2[:,ci] + bv2[ci]).T
            o_t = sb.tile([A, D], f32)

            nc.sync.dma_start(b1_t[:], b1_in[:])
            # Interleave per-chunk loads so stage-1 matmul c can start
            # as soon as its two chunks land.
            for c in range(KC):
                nc.sync.dma_start(
                    w1_t[:, c * 128:(c + 1) * 128], w1_in[:, c * 128:(c + 1) * 128]
                )
                nc.sync.dma_start(a_t[:, c * A:(c + 1) * A], a_in[:, c * A:(c + 1) * A])
            # w2 only gates stage 2; halves so matmul j starts at half-arrival.
            for j in range(2):
                nc.sync.dma_start(
                    w2_t[:, j * 512:(j + 1) * 512], w2_in[:, j * 512:(j + 1) * 512]
                )

            # Stage 1: v2t[128, A] = sum_c Wv2_chunk[c].T @ actT_chunk[c]
            p1 = ps.tile([128, A], f32)
            for c in range(KC):
                nc.tensor.matmul(
                    p1[:],
                    w1_t[:, c * 128:(c + 1) * 128],   # lhsT [K=128, M=128]
                    a_t[:, c * A:(c + 1) * A],        # rhs  [K=128, N=A]
                    start=(c == 0),
                    stop=(c == KC - 1),
                )
            # bias add (per-partition scalar) + PSUM->SBUF move in one DVE op
            nc.vector.tensor_scalar_add(v2t[:], p1[:], b1_t[:, 0:1])

            # Stage 2: partial[A, D] = v2t.T @ Wo2[ci, :]
            for j in range(2):
                p2 = ps.tile([A, 512], f32, tag=f"p2_{j}")
                nc.tensor.matmul(
                    p2[:],
                    v2t[:],                            # lhsT [K=128, M=A]
                    w2_t[:, j * 512:(j + 1) * 512],    # rhs  [K=128, N=512]
                    start=True,
                    stop=True,
                )
                nc.vector.tensor_copy(o_t[:, j * 512:(j + 1) * 512], p2[:])

            nc.sync.dma_start(out_d[:], o_t[:])

    nc.finalize()
    return nc


def _prep_in_maps(act, Wv2, bv2, Wo2):
    """Host-side sharding + layout prep (cheap: <2MB of numpy copies)."""
    actT = np.ascontiguousarray(act.T)                      # [D, A]
    a_host = np.ascontiguousarray(
        actT.reshape(KC, 128, A).transpose(1, 0, 2).reshape(128, KC * A)
    )
    in_maps = []
    for i in range(NCORES):
        sl = slice(CHUNK * i, CHUNK * (i + 1))
        w1 = np.ascontiguousarray(
            Wv2[:, sl].reshape(KC, 128, CHUNK).transpose(1, 0, 2).reshape(128, KC * CHUNK)
        )
        in_maps.append({
            "a_in": a_host,
            "w1_in": w1,
            "w2_in": np.ascontiguousarray(Wo2[sl, :]),
            "b1_in": np.ascontiguousarray(bv2[sl].reshape(128, 1)),
        })
    return in_maps


def run(act, Wv2, bv2, Wo2, bo2, trace=False):
    from concourse.bass_utils import run_bass_kernel_spmd

    if "nc" not in _CACHE:
        _CACHE["nc"] = _build()
    in_maps = _prep_in_maps(act, Wv2, bv2, Wo2)
    res = run_bass_kernel_spmd(
        _CACHE["nc"], in_maps, core_ids=list(range(NCORES)), trace=trace
    )
    partials = [np.asarray(r["out"], np.float32) for r in res.results]
    out = partials[0].copy()
    for p in partials[1:]:
        out += p
    out += bo2[None, :]
    return out, res


def kernel(comb_fea, action_fea, params):
    act = np.asarray(action_fea, np.float32)[0]             # [A, D]
    Wv2 = np.asarray(params["Wv2"], np.float32)
    bv2 = np.asarray(params["bv2"], np.float32)
    Wo2 = np.asarray(params["Wo2"], np.float32)
    bo2 = np.asarray(params["bo2"], np.float32)
    out, _ = run(act, Wv2, bv2, Wo2, bo2, trace=False)
    return out


# revision 2
# speedup vs baseline: 1.5418x; 1.5418x over previous
"""Trainium2 kernel for nn_ActionPrompt.

Mathematical simplification of the reference model: both softmaxes are
taken over a length-1 axis (kv length 1), so their outputs are
identically 1.0.  That makes the entire stage-1 attention and the
stage-2 score computation dead code, and the output reduces exactly to

    out = (act @ Wv2 + bv2) @ Wo2 + bo2        # [A=50, D=1024]

independent of comb_fea.  (The reference's final mean over the
num_comb axis averages 1024 identical copies of o2.)

Distribution over 8 NeuronCores: shard the intermediate dimension
D=1024 into 8 chunks of 128.  Core i holds Wv2[:, ci] (col shard) and
Wo2[ci, :] (row shard) plus the full (replicated) act, computes

    partial_i = (act @ Wv2[:, ci] + bv2[ci]) @ Wo2[ci, :]   # [50, 1024]

and the host unshards by summing the 8 partials and adding bo2 (the
all-reduce of per-shard partial sums from the sharding hint, performed
at gather time — an on-device collective's ~10us floor would dominate
this tiny problem).

Implementation notes:
 - raw Bass (no TileContext): the kernel is a tiny static DAG; manual
   semaphores avoid Tile's prologue + drain/barrier tail (~10us).
 - matmul operands in bf16 (PE runs 1 cycle/row; halves DMA bytes);
   PSUM accumulation is fp32.  rel err vs the f32 reference ~2e-3.
 - inputs are host-packed into 2+1 contiguous blobs so only 4 input
   DMAs are issued (HWDGE issue costs ~600ns each on the sequencer),
   split across the two HWDGE rings (sync + scalar).
"""

import numpy as np

A = 50      # num actions
D = 1024    # embed dim
NCORES = 8
CHUNK = D // NCORES  # 128 cols/rows of the intermediate dim per core

_CACHE = {}


def _build():
    import concourse.bacc as bacc
    import concourse.mybir as mybir

    f32 = mybir.dt.float32
    bf16 = mybir.dt.bfloat16

    nc = bacc.Bacc("TRN2")

    # blob1/blob2 per-partition layout (bf16):
    #   [ w1 c0 | w1 c1 | w1 c2 | w1 c3 | a c0 | a c1 | a c2 | a c3 ]
    #     128*4 = 512 elems              50*4 = 200 elems
    # where w1 chunk c partition p = Wv2[128c+p, core_cols],
    #       a  chunk c partition p = act[:, 128c+p]  (i.e. actT row).
    # blob2 is the same for c = 4..7.
    BL = 4 * CHUNK + 4 * A  # 712
    blob1_d = nc.dram_tensor("blob1", [128, BL], bf16, kind="ExternalInput")
    blob2_d = nc.dram_tensor("blob2", [128, BL], bf16, kind="ExternalInput")
    w2_d = nc.dram_tensor("w2b", [128, D], bf16, kind="ExternalInput")
    b1_d = nc.dram_tensor("b1", [128, 1], f32, kind="ExternalInput")
    out_d = nc.dram_tensor("out", [A, D], f32, kind="ExternalOutput")

    L1 = nc.alloc_sbuf_tensor("L1", [128, BL], bf16)
    L2 = nc.alloc_sbuf_tensor("L2", [128, BL], bf16)
    W2 = nc.alloc_sbuf_tensor("W2", [128, D], bf16)
    B1 = nc.alloc_sbuf_tensor("B1", [128, 1], f32)
    V2T = nc.alloc_sbuf_tensor("V2T", [128, A], bf16)
    OT = nc.alloc_sbuf_tensor("OT", [A, D], f32)

    P1 = nc.alloc_psum_tensor("P1", [128, A], f32)
    P2a = nc.alloc_psum_tensor("P2a", [A, 512], f32)
    P2b = nc.alloc_psum_tensor("P2b", [A, 512], f32)

    s_ld1 = nc.alloc_semaphore("s_ld1")
    s_ld2 = nc.alloc_semaphore("s_ld2")
    s_w2 = nc.alloc_semaphore("s_w2")
    s_b1 = nc.alloc_semaphore("s_b1")
    s_mm1 = nc.alloc_semaphore("s_mm1")
    s_v2 = nc.alloc_semaphore("s_v2")
    s_mm2 = nc.alloc_semaphore("s_mm2")
    s_cp = nc.alloc_semaphore("s_cp")
    s_out = nc.alloc_semaphore("s_out")

    with nc.Block(no_gpsimd_drain=True) as block:

        @block.sync
        def _(sync):
            sync.dma_start(L1[:], blob1_d[:]).then_inc(s_ld1, 16)
            sync.dma_start(L2[:], blob2_d[:]).then_inc(s_ld2, 16)
            sync.dma_start(W2[:], w2_d[:]).then_inc(s_w2, 16)

        @block.scalar
        def _(scalar):
            scalar.dma_start(B1[:], b1_d[:]).then_inc(s_b1, 16)
            scalar.wait_ge(s_cp, 1)
            scalar.dma_start(out_d[:, 0:512], OT[:, 0:512]).then_inc(s_out, 16)
            scalar.wait_ge(s_cp, 2)
            scalar.dma_start(out_d[:, 512:1024], OT[:, 512:1024]).then_inc(s_out, 16)
            scalar.wait_ge(s_out, 32)

        @block.tensor
        def _(tensor):
            tensor.wait_ge(s_ld1, 16)
            for c in range(4):
                nc.tensor.matmul(
                    P1[:],
                    L1[:, c * CHUNK:(c + 1) * CHUNK],
                    L1[:, 4 * CHUNK + c * A: 4 * CHUNK + (c + 1) * A],
                    start=(c == 0),
                    stop=False,
                )
            tensor.wait_ge(s_ld2, 16)
            for c in range(4):
                mm = nc.tensor.matmul(
                    P1[:],
                    L2[:, c * CHUNK:(c + 1) * CHUNK],
                    L2[:, 4 * CHUNK + c * A: 4 * CHUNK + (c + 1) * A],
                    start=False,
                    stop=(c == 3),
                )
            mm.then_inc(s_mm1, 1)
            tensor.wait_ge(s_v2, 1)
            tensor.wait_ge(s_w2, 16)
            nc.tensor.matmul(
                P2a[:], V2T[:], W2[:, 0:512], start=True, stop=True
            ).then_inc(s_mm2, 1)
            nc.tensor.matmul(
                P2b[:], V2T[:], W2[:, 512:1024], start=True, stop=True
            ).then_inc(s_mm2, 1)

        @block.vector
        def _(vector):
            vector.wait_ge(s_mm1, 1)
            vector.wait_ge(s_b1, 16)
            nc.vector.tensor_scalar_add(V2T[:], P1[:], B1[:, 0:1]).then_inc(s_v2, 1)
            vector.wait_ge(s_mm2, 1)
            nc.vector.tensor_copy(OT[:, 0:512], P2a[:]).then_inc(s_cp, 1)
            vector.wait_ge(s_mm2, 2)
            nc.vector.tensor_copy(OT[:, 512:1024], P2b[:]).then_inc(s_cp, 1)

    nc.finalize()
    return nc


def _prep_in_maps(act, Wv2, bv2, Wo2):
    """Host-side sharding + blob packing (cheap: ~3MB of numpy copies)."""
    import ml_dtypes

    bf = ml_dtypes.bfloat16
    actT = np.ascontiguousarray(act.T)  # [D, A]
    # a chunks [128, 4*A] per half, replicated to all cores
    a_half = [
        actT[h * 512:(h + 1) * 512, :].reshape(4, 128, A)
        .transpose(1, 0, 2).reshape(128, 4 * A).astype(bf)
        for h in range(2)
    ]
    in_maps = []
    for i in range(NCORES):
        sl = slice(CHUNK * i, CHUNK * (i + 1))
        w1 = Wv2[:, sl]  # [D, 128]
        w1_half = [
            w1[h * 512:(h + 1) * 512, :].reshape(4, 128, CHUNK)
            .transpose(1, 0, 2).reshape(128, 4 * CHUNK).astype(bf)
            for h in range(2)
        ]
        in_maps.append({
            "blob1": np.ascontiguousarray(
                np.concatenate([w1_half[0], a_half[0]], axis=1)
            ),
            "blob2": np.ascontiguousarray(
                np.concatenate([w1_half[1], a_half[1]], axis=1)
            ),
            "w2b": np.ascontiguousarray(Wo2[sl, :].astype(bf)),
            "b1": np.ascontiguousarray(bv2[sl].reshape(128, 1)),
        })
    return in_maps


def run(act, Wv2, bv2, Wo2, bo2, trace=False):
    from concourse.bass_utils import run_bass_kernel_spmd

    if "nc" not in _CACHE:
        _CACHE["nc"] = _build()
    in_maps = _prep_in_maps(act, Wv2, bv2, Wo2)
    res = run_bass_kernel_spmd(
        _CACHE["nc"], in_maps, core_ids=list(range(NCORES)), trace=trace
    )
    partials = [np.asarray(r["out"], np.float32) for r in res.results]
    out = partials[0].copy()
    for p in partials[1:]:
        out += p
    out += bo2[None, :]
    return out, res


def kernel(comb_fea, action_fea, params):
    act = np.asarray(action_fea, np.float32)[0]             # [A, D]
    Wv2 = np.asarray(params["Wv2"], np.float32)
    bv2 = np.asarray(params["bv2"], np.float32)
    Wo2 = np.asarray(params["Wo2"], np.float32)
    bo2 = np.asarray(params["bo2"], np.float32)
    out, _ = run(act, Wv2, bv2, Wo2, bo2, trace=False)
    return out


# revision 4
# speedup vs baseline: 1.9221x; 1.2467x over previous
"""Trainium2 kernel for nn_ActionPrompt.

Mathematical simplification of the reference model: both softmaxes are
taken over a length-1 axis (kv length 1), so their outputs are
identically 1.0.  That makes the entire stage-1 attention and the
stage-2 score computation dead code, and the output reduces exactly to

    out = (act @ Wv2 + bv2) @ Wo2 + bo2        # [A=50, D=1024]

independent of comb_fea.  (The reference's final mean over the
num_comb axis averages 1024 identical copies of o2.)

Distribution over 8 NeuronCores: shard the intermediate dimension
D=1024 into 8 chunks of 128.  Core i holds Wv2[:, ci] (col shard) and
Wo2[ci, :] (row shard) plus the full (replicated) act, computes

    partial_i = (act @ Wv2[:, ci] + bv2[ci]) @ Wo2[ci, :]   # [50, 1024]

and the host unshards by summing the 8 partials and adding bo2 (the
all-reduce of per-shard partial sums from the sharding hint, performed
at gather time — an on-device collective's ~10us floor would dominate
this tiny problem).

Implementation notes:
 - raw Bass (no TileContext): the kernel is a tiny static DAG; manual
   semaphores avoid Tile's prologue + drain/barrier machinery.
 - matmul operands in bf16 (PE runs 1 cycle/row; halves DMA bytes);
   PSUM accumulation is fp32.  rel err vs the f32 reference ~3e-3.
 - the bv2 bias is folded into the stage-1 PSUM accumulation as a
   K=1 matmul (bias row x ones row), so no separate bias DMA/op.
 - inputs are host-packed into 3 contiguous blobs -> 3 input DMAs,
   issued across the two HWDGE rings (sync + scalar, ~600ns issue
   cost each); output completion is guaranteed by the Block-exit
   per-engine drains, so no trailing wait is on the critical path.
"""

import numpy as np

A = 50      # num actions
D = 1024    # embed dim
NCORES = 8
CHUNK = D // NCORES  # 128 cols/rows of the intermediate dim per core

# blob1 free-dim layout (bf16):
#   [0:512)    w1 chunks c=0..3   (chunk c partition p = Wv2[128c+p, cols_i])
#   [512:712)  a  chunks c=0..3   (chunk c partition p = act[:, 128c+p])
#   [712:840)  partition 0: bv2[cols_i], others zero   (bias row, K=1 lhsT)
#   [840:890)  partition 0: ones, others zero          (ones row, K=1 rhs)
# blob2 = w1/a chunks c=4..7, free dim [0:712).
BL1 = 4 * CHUNK + 4 * A + CHUNK + A  # 890
BL2 = 4 * CHUNK + 4 * A              # 712

_CACHE = {}


def _build():
    import concourse.bacc as bacc
    import concourse.mybir as mybir

    f32 = mybir.dt.float32
    bf16 = mybir.dt.bfloat16

    nc = bacc.Bacc("TRN2")

    blob1_d = nc.dram_tensor("blob1", [128, BL1], bf16, kind="ExternalInput")
    blob2_d = nc.dram_tensor("blob2", [128, BL2], bf16, kind="ExternalInput")
    w2_d = nc.dram_tensor("w2b", [128, D], bf16, kind="ExternalInput")
    out_d = nc.dram_tensor("out", [A, D], f32, kind="ExternalOutput")

    L1 = nc.alloc_sbuf_tensor("L1", [128, BL1], bf16)
    L2 = nc.alloc_sbuf_tensor("L2", [128, BL2], bf16)
    W2 = nc.alloc_sbuf_tensor("W2", [128, D], bf16)
    V2T = nc.alloc_sbuf_tensor("V2T", [128, A], bf16)
    OT = nc.alloc_sbuf_tensor("OT", [A, D], f32)

    P1 = nc.alloc_psum_tensor("P1", [128, A], f32)
    P2a = nc.alloc_psum_tensor("P2a", [A, 512], f32)
    P2b = nc.alloc_psum_tensor("P2b", [A, 512], f32)

    s_ld1 = nc.alloc_semaphore("s_ld1")
    s_ld2 = nc.alloc_semaphore("s_ld2")
    s_w2 = nc.alloc_semaphore("s_w2")
    s_mm1 = nc.alloc_semaphore("s_mm1")
    s_v2 = nc.alloc_semaphore("s_v2")
    s_mm2 = nc.alloc_semaphore("s_mm2")
    s_cp = nc.alloc_semaphore("s_cp")
    s_out = nc.alloc_semaphore("s_out")  # completion tracked by Block-exit drains

    with nc.Block(no_gpsimd_drain=True) as block:

        @block.sync
        def _(sync):
            sync.dma_start(L1[:], blob1_d[:]).then_inc(s_ld1, 16)
            sync.dma_start(W2[:], w2_d[:]).then_inc(s_w2, 16)
            sync.wait_ge(s_cp, 2)
            sync.dma_start(out_d[:, 512:1024], OT[:, 512:1024]).then_inc(s_out, 16)

        @block.scalar
        def _(scalar):
            scalar.dma_start(L2[:], blob2_d[:]).then_inc(s_ld2, 16)
            scalar.wait_ge(s_cp, 1)
            scalar.dma_start(out_d[:, 0:512], OT[:, 0:512]).then_inc(s_out, 16)

        @block.tensor
        def _(tensor):
            tensor.wait_ge(s_ld1, 16)
            for c in range(4):
                nc.tensor.matmul(
                    P1[:],
                    L1[:, c * CHUNK:(c + 1) * CHUNK],
                    L1[:, 4 * CHUNK + c * A: 4 * CHUNK + (c + 1) * A],
                    start=(c == 0),
                    stop=False,
                )
            tensor.wait_ge(s_ld2, 16)
            for c in range(4):
                nc.tensor.matmul(
                    P1[:],
                    L2[:, c * CHUNK:(c + 1) * CHUNK],
                    L2[:, 4 * CHUNK + c * A: 4 * CHUNK + (c + 1) * A],
                    start=False,
                    stop=False,
                )
            # bias: P1[m, n] += bv2[m] * 1  (K=1 matmul from partition 0)
            nc.tensor.matmul(
                P1[:],
                L1[0:1, 712:712 + CHUNK],
                L1[0:1, 712 + CHUNK:712 + CHUNK + A],
                start=False,
                stop=True,
            ).then_inc(s_mm1, 1)
            tensor.wait_ge(s_v2, 1)
            tensor.wait_ge(s_w2, 16)
            nc.tensor.matmul(
                P2a[:], V2T[:], W2[:, 0:512], start=True, stop=True
            ).then_inc(s_mm2, 1)
            nc.tensor.matmul(
                P2b[:], V2T[:], W2[:, 512:1024], start=True, stop=True
            ).then_inc(s_mm2, 1)

        @block.vector
        def _(vector):
            vector.wait_ge(s_mm1, 1)
            nc.vector.tensor_copy(V2T[:], P1[:]).then_inc(s_v2, 1)
            vector.wait_ge(s_mm2, 1)
            nc.vector.tensor_copy(OT[:, 0:512], P2a[:]).then_inc(s_cp, 1)
            vector.wait_ge(s_mm2, 2)
            nc.vector.tensor_copy(OT[:, 512:1024], P2b[:]).then_inc(s_cp, 1)

    nc.finalize()
    return nc


def _prep_in_maps(act, Wv2, bv2, Wo2):
    """Host-side sharding + blob packing (cheap: ~3MB of numpy copies)."""
    import ml_dtypes

    bf = ml_dtypes.bfloat16
    actT = np.ascontiguousarray(act.T)  # [D, A]
    a_half = [
        actT[h * 512:(h + 1) * 512, :].reshape(4, 128, A)
        .transpose(1, 0, 2).reshape(128, 4 * A).astype(bf)
        for h in range(2)
    ]
    ones_pad = np.zeros((128, CHUNK + A), dtype=bf)
    in_maps = []
    for i in range(NCORES):
        sl = slice(CHUNK * i, CHUNK * (i + 1))
        w1 = Wv2[:, sl]  # [D, 128]
        w1_half = [
            w1[h * 512:(h + 1) * 512, :].reshape(4, 128, CHUNK)
            .transpose(1, 0, 2).reshape(128, 4 * CHUNK).astype(bf)
            for h in range(2)
        ]
        tailpad = ones_pad.copy()
        tailpad[0, :CHUNK] = bv2[sl].astype(bf)
        tailpad[0, CHUNK:] = bf(1.0)
        in_maps.append({
            "blob1": np.ascontiguousarray(
                np.concatenate([w1_half[0], a_half[0], tailpad], axis=1)
            ),
            "blob2": np.ascontiguousarray(
                np.concatenate([w1_half[1], a_half[1]], axis=1)
            ),
            "w2b": np.ascontiguousarray(Wo2[sl, :].astype(bf)),
        })
    return in_maps


def run(act, Wv2, bv2, Wo2, bo2, trace=False):
    from concourse.bass_utils import run_bass_kernel_spmd

    if "nc" not in _CACHE:
        _CACHE["nc"] = _build()
    in_maps = _prep_in_maps(act, Wv2, bv2, Wo2)
    res = run_bass_kernel_spmd(
        _CACHE["nc"], in_maps, core_ids=list(range(NCORES)), trace=trace
    )
    partials = [np.asarray(r["out"], np.float32) for r in res.results]
    out = partials[0].copy()
    for p in partials[1:]:
        out += p
    out += bo2[None, :]
    return out, res


def kernel(comb_fea, action_fea, params):
    act = np.asarray(action_fea, np.float32)[0]             # [A, D]
    Wv2 = np.asarray(params["Wv2"], np.float32)
    bv2 = np.asarray(params["bv2"], np.float32)
    Wo2 = np.asarray(params["Wo2"], np.float32)
    bo2 = np.asarray(params["bo2"], np.float32)
    out, _ = run(act, Wv2, bv2, Wo2, bo2, trace=False)
    return out
